# revision 15
# baseline (speedup 1.0000x reference)
"""Sparse (sliding-window) attention head on 8 TRN2 NeuronCores.

Reference computation (B=2, S=4096, D=512, HD=64, SCALE=128):
    q = x @ wq ; k = x @ wk ; v = x @ wv          [B,S,64]
    scores[b,s,w] = q[b,s] . k[b,s-128+w] / 8     w in [0,256), zero-padded OOB
    out = softmax_w(scores) @ v_window            [B,S,64]

Sharding: 8 shards = (batch b, 1024-seq chunk c). Each shard gets a
zero-padded 128-halo of x on both sides, which reproduces the reference's
zero-padded (not masked) window semantics exactly. All compute is local.

v2 layout (per core):
    x arrives as 4 column-block tiles [128, 4dc, w] (bf16, host-transposed),
    DMA'd in consumption order on both HWDGE engines (Sync + Scalar).
    wq|wk packed into one [128,128] lhsT per d-chunk -> one PE pass emits
    qT (rows 0:64) and kT (rows 64:128) together into qkT [128, 1280].
    v stays natural [key,64] via per-chunk matmuls, packed pairwise in PSUM.
    Attention per 128-query block qb (starts as soon as its 384-col window
    of kT/qT/v is evacuated):
        scT[key,que] = kT_chunk.T @ qT_block      3 chunks of [128,128]
        ex = exp(scT/8) (ACT), em = ex*mask (DVE/Pool alternating)
        av4[:, qb%4, 0:65] += em_c.T @ vaug_c     ones col -> softmax denom
        norm: recip batched per 4 blocks, out_block = av*(1/denom)
    Output [128, 8, 64] partition-major, 2 DMAs; host unshuffles.
"""

import sys
import types

import numpy as np
import ml_dtypes

B, S, D = 2, 4096, 512
HD = 64
SCALE = 128
SS = S // 4          # 1024 positions per shard
HP = SCALE           # halo padding each side
NP = SS + 2 * HP     # 1280 padded positions
NKC = NP // 128      # 10 key chunks
NQB = SS // 128      # 8 query blocks
NDC = D // 128       # 4 d-chunks

# x column blocks (multiples of 128; qk proj segments == blocks)
BLOCKS = [(0, 128), (128, 128), (256, 256), (512, 256), (768, 256), (1024, 256)]

_CACHE = {}


def _ensure_hooks():
    """Register the axon NTFF profile hook; keep artifacts local."""
    if "antenv.axon_hooks" not in sys.modules:
        try:
            from trn_agent_boot.trn_boot import _ntff_profile_via_ctypes

            m = types.ModuleType("antenv.axon_hooks")
            m.get_axon_ntff_profile_hook = lambda: _ntff_profile_via_ctypes(
                "/opt/axon/libaxon_pjrt.so"
            )
            sys.modules["antenv.axon_hooks"] = m
        except Exception:
            pass
    import concourse.bass_utils as bass_utils

    bass_utils.upload_artifacts = lambda tmpdir: tmpdir


def _build_nc():
    import concourse.mybir as mybir
    import concourse.tile as tile
    from concourse import bacc

    bf = mybir.dt.bfloat16
    f32 = mybir.dt.float32
    AF = mybir.ActivationFunctionType

    nc = bacc.Bacc("TRN2", target_bir_lowering=False, debug=False, num_devices=8)

    xb_d = [
        nc.dram_tensor(f"xb{bi}", [128, NDC, w], bf, kind="ExternalInput")
        for bi, (s0, w) in enumerate(BLOCKS)
    ]
    w_d = nc.dram_tensor("wqkv", [128, NDC, 192], bf, kind="ExternalInput")
    mask_d = nc.dram_tensor("mask", [128, 2, 128], bf, kind="ExternalInput")
    out_d = nc.dram_tensor("outp", [128, NQB, HD], f32, kind="ExternalOutput")

    # chunk -> block containing its 128 cols
    def blk_of(col):
        for bi, (s0, w) in enumerate(BLOCKS):
            if s0 <= col < s0 + w:
                return bi, col - s0
        raise AssertionError(col)

    with tile.TileContext(nc) as tc:
        with (
            tc.tile_pool(name="consts", bufs=1) as consts,
            tc.tile_pool(name="xtp", bufs=1) as xtp,
            tc.tile_pool(name="qkp", bufs=1) as qkp,
            tc.tile_pool(name="vgp", bufs=1) as vgp,
            tc.tile_pool(name="exp_p", bufs=2) as exp_p,
            tc.tile_pool(name="emp", bufs=2) as emp,
            tc.tile_pool(name="fin", bufs=2) as fin,
            tc.tile_pool(name="qkps", bufs=2, space="PSUM") as qkps,
            tc.tile_pool(name="vps", bufs=2, space="PSUM") as vps,
            tc.tile_pool(name="scps", bufs=2, space="PSUM") as scps,
            tc.tile_pool(name="avps", bufs=2, space="PSUM") as avps,
        ):
            # ---- DMAs first, split across both HWDGE engines ----
            xt = []
            for bi, (s0, w) in enumerate(BLOCKS):
                t = xtp.tile([128, NDC, w], bf, tag=f"xt{bi}")
                xt.append(t)
            w_s = consts.tile([128, NDC, 192], bf, tag="w")
            mask_s = consts.tile([128, 2, 128], bf, tag="mask")

            def dma_blk(eng, bi):
                s0, w = BLOCKS[bi]
                eng.dma_start(out=xt[bi], in_=x_d[:, :, s0 : s0 + w])

            nc.sync.dma_start(out=w_s, in_=w_d[:, :, :])
            dma_blk(nc.scalar, 0)
            dma_blk(nc.sync, 1)
            dma_blk(nc.scalar, 2)
            nc.sync.dma_start(out=mask_s, in_=mask_d[:, :, :])
            dma_blk(nc.scalar, 3)
            dma_blk(nc.sync, 4)
            dma_blk(nc.scalar, 5)

            # ---- memsets + ACT exp-table trigger ----
            zz = consts.tile([128, 1], f32, tag="zz")
            nc.vector.memset(zz, 0.0)
            garb = consts.tile([128, 260], bf, tag="garb")
            nc.vector.memset(garb, 0.5)
            vaug = vgp.tile([128, NKC, 66], bf, tag="vaug")
            nc.gpsimd.memset(vaug[:, :, 64:66], 1.0)
            ez = consts.tile([128, 1], f32, tag="ez")
            nc.scalar.activation(ez, zz, AF.Exp)

            qT_s = qkp.tile([64, SS], bf, tag="qT")
            kT_s = qkp.tile([64, NP], bf, tag="kT")
            ot = fin.tile([128, NQB, HD], f32, tag="ot")

            # ---- PE warmup: ramp to full clock while DMAs land ----
            for i in range(12):
                wp = avps.tile([128, 4, 65], f32, tag="av4")
                nc.tensor.matmul(
                    wp[:, :, :],
                    lhsT=garb[:, 0:128],
                    rhs=garb[:, :],
                    start=True,
                    stop=True,
                )

            # ---- helpers ----
            def qk_seg(bi):
                s0, w = BLOCKS[bi]
                ps = qkps.tile([128, 256], f32, tag="qkps")
                for dc in range(NDC):
                    nc.tensor.matmul(
                        ps[:, :w],
                        lhsT=w_s[:, dc, 0:128],
                        rhs=xt[bi][:, dc, :],
                        start=(dc == 0),
                        stop=(dc == NDC - 1),
                    )
                return ps, s0, w

            def v_chunk(kc, vp, j):
                bi, off = blk_of(kc * 128)
                for dc in range(NDC):
                    nc.tensor.matmul(
                        vp[:, j, :],
                        lhsT=xt[bi][:, dc, off : off + 128],
                        rhs=w_s[:, dc, 128:192],
                        start=(dc == 0),
                        stop=(dc == NDC - 1),
                    )

            def evac_k(ps, s0, w, eng):
                cp = eng.copy if eng is nc.scalar else eng.tensor_copy
                cp(kT_s[:, s0 : s0 + w], ps[64:128, :w])

            def evac_q(ps, s0, w, eng):
                cp = eng.copy if eng is nc.scalar else eng.tensor_copy
                qa, qb_ = max(s0, HP), min(s0 + w, HP + SS)
                if qa < qb_:
                    cp(qT_s[:, qa - HP : qb_ - HP], ps[0:64, qa - s0 : qb_ - s0])

            def sc_block(qb):
                sc = scps.tile([128, 384], f32, tag="sc")
                for c in range(3):
                    nc.tensor.matmul(
                        sc[:, c * 128 : (c + 1) * 128],
                        lhsT=kT_s[:, (qb + c) * 128 : (qb + c + 1) * 128],
                        rhs=qT_s[:, qb * 128 : (qb + 1) * 128],
                        start=True,
                        stop=True,
                    )
                return sc

            def exp_block(sc):
                ex = exp_p.tile([128, 3, 128], bf, tag="ex")
                nc.scalar.activation(ex[:, :, :], sc, AF.Exp, scale=0.125)
                return ex

            def mask_block(ex, eng):
                # only outer chunks need the band mask; middle is all-valid
                em = emp.tile([128, 2, 128], bf, tag="em")
                eng.tensor_mul(em, ex[:, 0::2, :], mask_s)
                return em

            def av_block(qb, ex, em, av4, j):
                nc.tensor.matmul(
                    av4[:, j, :],
                    lhsT=ex[:, 1, :],
                    rhs=vaug[:, qb + 1, 0:65],
                    start=True,
                    stop=False,
                )
                nc.tensor.matmul(
                    av4[:, j, :],
                    lhsT=em[:, 0, :],
                    rhs=vaug[:, qb, 0:65],
                    start=False,
                    stop=False,
                )
                nc.tensor.matmul(
                    av4[:, j, :],
                    lhsT=em[:, 1, :],
                    rhs=vaug[:, qb + 2, 0:65],
                    start=False,
                    stop=True,
                )

            # ---- pipeline (emission order == engine priority order) ----
            av4a = avps.tile([128, 4, 65], f32, tag="av4")

            ps0, s0, w0 = qk_seg(0)
            evac_k(ps0, s0, w0, nc.scalar)
            ps1, s1, w1 = qk_seg(1)
            evac_k(ps1, s1, w1, nc.scalar)
            evac_q(ps1, s1, w1, nc.vector)
            ps2, s2, w2 = qk_seg(2)
            evac_k(ps2, s2, w2, nc.scalar)
            evac_q(ps2, s2, w2, nc.vector)

            sc0 = sc_block(0)
            ex0 = exp_block(sc0)

            vp01 = vps.tile([128, 2, HD], f32, tag="vp")
            v_chunk(0, vp01, 0)
            v_chunk(1, vp01, 1)
            nc.vector.tensor_copy(vaug[:, 0:2, 0:64], vp01)

            sc1 = sc_block(1)
            ex1 = exp_block(sc1)

            ps3, s3, w3 = qk_seg(3)
            evac_k(ps3, s3, w3, nc.vector)
            evac_q(ps3, s3, w3, nc.vector)
            em0 = mask_block(ex0, nc.vector)

            vp23 = vps.tile([128, 2, HD], f32, tag="vp")
            v_chunk(2, vp23, 0)
            v_chunk(3, vp23, 1)
            nc.vector.tensor_copy(vaug[:, 2:4, 0:64], vp23)

            sc2 = sc_block(2)
            ex2 = exp_block(sc2)
            em1 = mask_block(ex1, nc.gpsimd)

            vp45 = vps.tile([128, 2, HD], f32, tag="vp")
            v_chunk(4, vp45, 0)
            v_chunk(5, vp45, 1)
            nc.vector.tensor_copy(vaug[:, 4:6, 0:64], vp45)

            av_block(0, ex0, em0, av4a, 0)

            ps4, s4, w4 = qk_seg(4)
            evac_k(ps4, s4, w4, nc.vector)
            evac_q(ps4, s4, w4, nc.vector)

            sc3 = sc_block(3)
            ex3 = exp_block(sc3)
            em2 = mask_block(ex2, nc.gpsimd)
            av_block(1, ex1, em1, av4a, 1)

            vp67 = vps.tile([128, 2, HD], f32, tag="vp")
            v_chunk(6, vp67, 0)
            v_chunk(7, vp67, 1)
            nc.vector.tensor_copy(vaug[:, 6:8, 0:64], vp67)

            sc4 = sc_block(4)
            ex4 = exp_block(sc4)
            av_block(2, ex2, em2, av4a, 2)

            ps5, s5, w5 = qk_seg(5)
            evac_k(ps5, s5, w5, nc.vector)
            evac_q(ps5, s5, w5, nc.vector)

            vp89 = vps.tile([128, 2, HD], f32, tag="vp")
            v_chunk(8, vp89, 0)
            v_chunk(9, vp89, 1)
            nc.vector.tensor_copy(vaug[:, 8:10, 0:64], vp89)

            sc5 = sc_block(5)
            ex5 = exp_block(sc5)
            em3 = mask_block(ex3, nc.vector)
            av_block(3, ex3, em3, av4a, 3)

            # group0 head: recip + first two norms + first output DMA
            rc0 = fin.tile([128, 4], f32, tag="rc")
            nc.vector.reciprocal(rc0, av4a[:, :, 64])
            nc.vector.tensor_scalar_mul(ot[:, 0, :], av4a[:, 0, 0:64], rc0[:, 0:1])
            nc.vector.tensor_scalar_mul(ot[:, 1, :], av4a[:, 1, 0:64], rc0[:, 1:2])
            nc.sync.dma_start(out=out_d[:, 0:2, :], in_=ot[:, 0:2, :])

            av4b = avps.tile([128, 4, 65], f32, tag="av4")
            em4 = mask_block(ex4, nc.vector)
            sc6 = sc_block(6)
            ex6 = exp_block(sc6)
            av_block(4, ex4, em4, av4b, 0)

            em5 = mask_block(ex5, nc.vector)
            sc7 = sc_block(7)
            ex7 = exp_block(sc7)
            av_block(5, ex5, em5, av4b, 1)

            rc45 = fin.tile([128, 2], f32, tag="rc2")
            nc.vector.reciprocal(rc45, av4b[:, 0:2, 64])
            nc.vector.tensor_scalar_mul(ot[:, 4, :], av4b[:, 0, 0:64], rc45[:, 0:1])
            nc.vector.tensor_scalar_mul(ot[:, 5, :], av4b[:, 1, 0:64], rc45[:, 1:2])
            nc.sync.dma_start(out=out_d[:, 4:6, :], in_=ot[:, 4:6, :])

            em6 = mask_block(ex6, nc.vector)
            av_block(6, ex6, em6, av4b, 2)
            em7 = mask_block(ex7, nc.vector)
            av_block(7, ex7, em7, av4b, 3)

            rc6 = fin.tile([128, 1], f32, tag="rc1")
            nc.vector.reciprocal(rc6, av4b[:, 2, 64:65])
            nc.vector.tensor_scalar_mul(ot[:, 6, :], av4b[:, 2, 0:64], rc6)
            rc7 = fin.tile([128, 1], f32, tag="rc1")
            nc.vector.reciprocal(rc7, av4b[:, 3, 64:65])
            nc.vector.tensor_scalar_mul(ot[:, 7, :], av4b[:, 3, 0:64], rc7)
            nc.sync.dma_start(out=out_d[:, 6:8, :], in_=ot[:, 6:8, :])

            # remaining group0 norms + their DMA on ACT (after its exp chain)
            nc.scalar.activation(
                ot[:, 2, :], av4a[:, 2, 0:64], AF.Copy, scale=rc0[:, 2:3]
            )
            nc.scalar.activation(
                ot[:, 3, :], av4a[:, 3, 0:64], AF.Copy, scale=rc0[:, 3:4]
            )
            nc.scalar.dma_start(out=out_d[:, 2:4, :], in_=ot[:, 2:4, :])
    nc.compile()
    return nc


def _get_nc():
    if "nc" not in _CACHE:
        _ensure_hooks()
        _CACHE["nc"] = _build_nc()
    return _CACHE["nc"]


def _host_inputs(inputs, wq, wk, wv):
    bf16 = ml_dtypes.bfloat16
    x = np.asarray(inputs, dtype=np.float32)

    # wqkv[p, dc, 0:64]=wq, [64:128]=wk, [128:192]=wv  (rows dc*128+p)
    wcat = np.concatenate(
        [np.asarray(wq), np.asarray(wk), np.asarray(wv)], axis=1
    ).astype(np.float32)                                     # [512, 192]
    wqkv = np.ascontiguousarray(
        wcat.reshape(NDC, 128, 192).transpose(1, 0, 2)
    ).astype(bf16)                                           # [128, 4, 192]

    p = np.arange(128)[:, None]
    q = np.arange(128)[None, :]
    # outer-chunk validity only (middle chunk of the 384-span is all-valid)
    mask = np.stack([(p >= q), (p < q)], axis=1).astype(bf16)  # [128, 2, 128]

    in_maps = []
    for i in range(8):
        b, c = divmod(i, 4)
        s0 = c * SS
        xp = np.zeros((NP, D), np.float32)
        lo = max(0, s0 - HP)
        hi = min(S, s0 + SS + HP)
        xp[lo - (s0 - HP) : hi - (s0 - HP)] = x[b, lo:hi]
        x4 = xp.T.reshape(NDC, 128, NP).transpose(1, 0, 2)  # [128, 4, 1280]
        m = {"wqkv": wqkv, "mask": mask}
        for bi, (s0, w) in enumerate(BLOCKS):
            m[f"xb{bi}"] = np.ascontiguousarray(x4[:, :, s0 : s0 + w]).astype(bf16)
        in_maps.append(m)
    return in_maps


def run_sharded(inputs, wq, wk, wv, trace=False, trace_cores=None):
    """Run the SPMD kernel; returns (out [B,S,HD] f32, BassKernelResults)."""
    _ensure_hooks()
    import concourse.bass_utils as bass_utils

    nc = _get_nc()
    in_maps = _host_inputs(inputs, wq, wk, wv)
    res = bass_utils.run_bass_kernel_spmd(
        nc,
        in_maps,
        core_ids=list(range(8)),
        trace=trace,
        trace_cores=trace_cores,
    )
    out = np.empty((B, S, HD), np.float32)
    for i in range(8):
        b, c = divmod(i, 4)
        o = res.results[i]["outp"]                           # [128, 8, 64]
        out[b, c * SS : (c + 1) * SS] = o.transpose(1, 0, 2).reshape(SS, HD)
    return out, res


def kernel(inputs, wq, wk, wv):
    out, _ = run_sharded(inputs, wq, wk, wv, trace=False)
    return out
